# revision 104
# baseline (speedup 1.0000x reference)
"""Trainium2 Bass kernel for nn_AttentionModule_24068996726850.

Mathematical collapse: the reference expands [N, C] -> [N, C, L] with L
identical columns, so every [*, *, l] slice is identical.  The softmax
over L of constant logits is uniform (1/L), and sum_l attn*value
reduces to `value` itself.  The whole module is therefore:

    P   = relu(features  @ Wk.T)
    Hh  = relu(bn1(features2 @ Wv1.T))
    V   = relu(bn2(Hh @ Wv2.T))
    Cc  = sigmoid(P @ Wvc.T)
    out = V + P * Cc          # [N, 512]

(verified to ~7e-7 scale-relative against the full reference formula).
Wa / Wqk / key_e drop out entirely.

Device dataflow (per core, pure data-parallel over N):
  - activations kept transposed [512 channels x n] so the channel axis
    sits on SBUF partitions; BN folds into per-partition scale/bias of
    the eviction instruction; no on-chip transposes anywhere.
  - weights pre-transposed on host to [cin, cout] = lhsT layout.
  - H/V stages in bf16; P stage in fp8 DoubleRow: k0-1 as ONE pair-DR
    matmul (both operands single fp8), k2-3 as hi+lo-weight oneop-DR
    with a stride-0 rhs.  fp8 here is both an instruction-count cut
    (pair-DR packs 2 k-groups/matmul) and a POWER lever: fp8
    multipliers keep the PE out of the P0 downclock (2.4 vs 2.0 GHz
    measured).  C stage in fp8 pair-DoubleRow.
  - measured ~221us/core at rel err 1.68e-2 (gate 2e-2, deterministic
    inputs): PE ~85% busy at 2.4 GHz.  Pushing occupancy higher trips
    the power-state downclock and is net-negative.
"""

import numpy as np
from contextlib import ExitStack

N_CORES = 8
N_TOTAL = 65536
C = 512            # input channels
D = 512            # output channels
NS = N_TOTAL // N_CORES   # 8192 rows per core
NF = 512           # columns (rows of the original problem) per chunk
NCHUNK = NS // NF  # 16
KG = C // 128      # 4 contraction partition-groups
MG = D // 128      # 4 output-channel partition-groups
BN_EPS = 1e-5

MM_DTYPE = "bfloat16"   # matmul input dtype: float32r | float32 | bfloat16

# fp8 DoubleRow acceleration (0.5 cycles/output-row on the PE):
#   P_ONEOP: P-stage matmuls pair (hi, lo) fp8e4m3 splits of Wk in the
#     two DoubleRow k-planes against a stride-0-broadcast fp8 rhs:
#     (Whi+Wlo)@X in half the cycles with only the activation
#     quantization (~3.6%/elem RMS) as extra error.
#   C_FP8: C-stage matmuls run plain DoubleRow over k-group pairs with
#     both operands single fp8 (sigmoid compresses the error).
# Weights are pre-scaled x32 before fp8 quantization (keeps them out of
# e4m3's denormal range); the eviction activations divide by 32.
P_ONEOP = True    # cycle-neutral on HW (a DoubleRow MM streams columns at
                  # the same 1/cycle as bf16) but POWER-positive: fp8
                  # multipliers draw less, which keeps the PE out of the
                  # P0 downclock (2.4 GHz vs 2.0 GHz measured) — worth
                  # ~20% wall clock on every matmul in the kernel
C_FP8 = True
P_HALF = True     # k0-1 of P in ONE full-fp8 pair-DR matmul, k2-3 hi/lo
W8SCALE = 32.0
# Keep the stock semaphore-clearing teardown: it costs ~0 inside the
# measured span (the NRT epilogue overlaps it) and the NEFF stays safely
# re-executable if the caller runs the kernel more than once per load.
SLIM_TEARDOWN = False

_CACHE = {}


def _patch_tail_drain():
    """The kernel-tail drain emitted at TileContext exit carries one wait
    per logical proc (13 here) — far over walrus's one-sync-wait budget.
    Split it into per-proc drains with identical semantics."""
    import concourse.tile as tile
    from concourse.vector_clock import ScopedClock, VectorClock

    if getattr(tile.TileContext, "_tail_split_patched", False):
        return

    def _split(self, tick_clock, wait_clock):
        gc = tick_clock.global_clock
        n = len(gc)
        for p in range(n):
            t = gc[p]
            if t <= 0:
                continue
            vec = [0] * n
            vec[p] = t
            d = self.nc.sync.drain()
            wait_clock.add_sem_waits(
                d.ins, ScopedClock({None: VectorClock(vec)})
            )
        self.nc.all_engine_barrier()
        assert self.sems is not None
        popped = self.nc._tile_sem_poison_stack.pop()
        assert popped is self._sem_poison
        if SLIM_TEARDOWN:
            # The NEFF executes once per load, so the ~7us gpsimd
            # dma_reset + sem_clear pass (plus its closing barrier) only
            # protects a re-execution that never happens.  Keep the
            # host-side bookkeeping so TileContext exit stays coherent.
            sems = list(self.sems.allocated().values())
            sem_nums = [s.num if hasattr(s, "num") else s for s in sems]
            self.nc._state.prepend_free_semaphores(sem_nums)
            for poison_set in self.nc._tile_sem_poison_stack:
                poison_set.update(sem_nums)
        else:
            self.nc.clear_and_free_semaphores(
                list(self.sems.allocated().values()))
            self.nc.all_engine_barrier()

    tile.TileContext._drain_and_barrier = _split
    tile.TileContext._tail_split_patched = True


def _build_program():
    import concourse.bass as bass
    import concourse.mybir as mybir
    import concourse.tile as tile
    from concourse.bass import ds
    from concourse.tile import add_dep_helper

    FP32 = mybir.dt.float32
    MMDT = getattr(mybir.dt, MM_DTYPE)
    AF = mybir.ActivationFunctionType

    # HARD CONSTRAINT on this toolchain: walrus allows at most ONE
    # sync-wait per instruction.  Tile elides a wait only if this engine
    # already waited a >= tick of that semaphore via an earlier
    # DATA-dependent instruction (manual sync edges / Drains don't
    # count).  The kernel threads tiny observer ops through every engine
    # so each real instruction needs <= 1 wait:
    #   PE  <- ACT ticks: tiny matmuls reading evicted tiles
    #   ACT <- DVE ticks: tiny Copy activations
    #   DVE <- PE ticks:  write-once PSUM "mailbox" stamped by PE
    #   SP  <- DVE ticks: tiny SBUF->DRAM observer DMA
    # Activations bounce DRAM -> per-k landing tile -> DVE copy ->
    # compute tile so a landing slot's accessor set is {one DMA, one DVE
    # copy} and its reuse wait collapses to one DMA-queue semaphore.
    # All DRAM tensors are chunk-major so per-chunk accesses are
    # disjoint regions (no conservative overlap waits).

    _patch_tail_drain()
    nc = bass.Bass()

    FP8 = mybir.dt.float8e4
    FT_DT = FP8 if P_ONEOP else MMDT
    WVC_DT = FP8 if C_FP8 else MMDT

    ft = nc.declare_dram_parameter("ft", [NCHUNK, 128, KG * NF], FT_DT,
                                   isOutput=False)
    f2t = nc.declare_dram_parameter("f2t", [NCHUNK, 128, KG * NF], MMDT,
                                    isOutput=False)
    if P_ONEOP:
        wkhi = nc.declare_dram_parameter("wkhi", [C, D], FP8, isOutput=False)
        wklo = nc.declare_dram_parameter("wklo", [C, D], FP8, isOutput=False)
        if P_HALF:
            wk8 = nc.declare_dram_parameter("wk8", [C // 2, D], FP8,
                                            isOutput=False)
    else:
        wk = nc.declare_dram_parameter("wk", [C, D], MMDT, isOutput=False)
    wv1 = nc.declare_dram_parameter("wv1", [C, D], MMDT, isOutput=False)
    wv2 = nc.declare_dram_parameter("wv2", [D, D], MMDT, isOutput=False)
    wvc = nc.declare_dram_parameter("wvc", [D, D], WVC_DT, isOutput=False)
    # [bn1s | bn1b | bn2s | bn2b | zeros] in one tensor -> one DMA
    bnv = nc.declare_dram_parameter("bnv", [128, 5 * MG + 128], FP32,
                                    isOutput=False)
    # one output tensor per chunk: DRAM dep tracking is tensor-granular,
    # so a shared output tensor would chain every out-DMA via WAW waits
    outs = [
        nc.declare_dram_parameter(f"out{j}", [D, NF], MMDT, isOutput=True)
        for j in range(NCHUNK)
    ]

    with tile.TileContext(nc) as tc:
        with ExitStack() as ctx:
            consts = ctx.enter_context(tc.tile_pool(name="consts", bufs=1))

            def load_weight(dram, dt=None):
                t = consts.tile([128, KG, D], dt or MMDT,
                                tag=f"w_{dram.name}")
                nc.scalar.dma_start(
                    t[:], dram[:].rearrange("(k p) o -> p k o", p=128))
                return t

            wk8_t = None
            if P_ONEOP:
                # (hi, lo) fp8 planes of 32*Wk^T for DoubleRow lhsT
                wk_t = consts.tile([128, KG, 2, D], FP8, tag="w_wk")
                if P_HALF:
                    wk8_t = consts.tile([128, 2, D], FP8, tag="w_wk8")
                    nc.scalar.dma_start(
                        wk8_t[:],
                        wk8[:].rearrange("(k p) o -> p k o", p=128))
                nc.scalar.dma_start(
                    wk_t[:, :, 0, :],
                    wkhi[:].rearrange("(k p) o -> p k o", p=128))
                nc.scalar.dma_start(
                    wk_t[:, :, 1, :],
                    wklo[:].rearrange("(k p) o -> p k o", p=128))
            else:
                wk_t = load_weight(wk)

            # startup DMA order follows chunk-0 stage needs: wk, chunk-0
            # activations (P can start), wv1 (H), wvc (C), wv2 (V), then
            # chunk-1 activations — DMA bandwidth is the startup critical
            # path and this ramp lets each stage begin as its data lands
            io_pool_early = ctx.enter_context(tc.tile_pool(name="ioe", bufs=2))
            early_loads = {}
            HLF = KG * NF // 2

            def early_load(j0):
                ftX0 = io_pool_early.tile([128, KG * NF], FT_DT, tag="ftX",
                                          name=f"ftXe_{j0}")
                f2tX0 = io_pool_early.tile([128, KG * NF], MMDT, tag="f2tX",
                                           name=f"f2tXe_{j0}")
                nc.scalar.dma_start(ftX0[:, 0:HLF], ft[j0, :, 0:HLF])
                nc.scalar.dma_start(ftX0[:, HLF:], ft[j0, :, HLF:])
                nc.scalar.dma_start(f2tX0[:, 0:HLF], f2t[j0, :, 0:HLF])
                nc.scalar.dma_start(f2tX0[:, HLF:], f2t[j0, :, HLF:])
                early_loads[j0] = (ftX0, f2tX0)

            # write-once scratch columns for observer writes
            scrA = consts.tile([128, 3 * NCHUNK + 8], FP32, tag="scrA")
            scrD = consts.tile([128, 3 * NCHUNK + 16], FP32, tag="scrD")
            STB = 2 * NCHUNK + 10

            # chunk 0 is the startup critical path: land each tensor in
            # k-group quarters interleaved with the weight loads in stage
            # order (wk, ft -> P; wv1, f2t -> H; wvc -> C; wv2 -> V) so
            # the first P matmul starts after wk + 1/4 of ft arrives
            ftX0 = io_pool_early.tile([128, KG * NF], FT_DT, tag="ftX",
                                      name="ftXe_0")
            f2tX0 = io_pool_early.tile([128, KG * NF], MMDT, tag="f2tX",
                                       name="f2tXe_0")
            # halves, not quarters: the P_HALF pair-DR matmul reads
            # ft[0:2NF] in one instruction, and a quarter-split would
            # give it two DMA-queue waits (over walrus's one-wait budget)
            nc.scalar.dma_start(ftX0[:, 0:HLF], ft[0, :, 0:HLF])
            nc.scalar.dma_start(ftX0[:, HLF:], ft[0, :, HLF:])

            # bnv (76KB of BN scale/bias) right after ft0, and its ACT +
            # DVE observers HERE in the stream: the first Activation
            # instruction triggers the one-time ~1.3us ACT_TABLE_LOAD, so
            # placing it here lets the table load overlap the ft0 data
            # flight.  At the old position (observers emitted inside the
            # j==0 loop body, bnv loaded last) the table load + bnv wait
            # blocked every eviction until ~22us — a 4.1us PE stall in
            # the middle of chunk 0.
            bnv_t = consts.tile([128, 5 * MG + 128], FP32, tag="bnv")
            nc.scalar.dma_start(bnv_t[:], bnv[:])
            ao_bnv = nc.scalar.activation(
                scrA[0:1, 0:1], bnv_t[0:1, 0:1], AF.Copy).ins
            anchor_bnv = nc.vector.tensor_copy(
                scrD[0:1, ds(2 * NCHUNK + 1, 1)], bnv_t[0:1, 0:1]).ins

            wv1_t = load_weight(wv1)
            # wvc (256KB fp8) right after wv1: queued behind f2t0 + ft1
            # it landed ~2us after the chunk-0 C matmuls wanted it
            wvc_t = load_weight(wvc, WVC_DT)
            for q in range(KG):
                nc.scalar.dma_start(f2tX0[:, q * NF:(q + 1) * NF],
                                    f2t[0, :, q * NF:(q + 1) * NF])
            early_loads[0] = (ftX0, f2tX0)
            # chunk 1's ft lands before wvc/wv2: those weights are not
            # read until ~7us into chunk 0, while chunk 1's P matmuls
            # start right at the chunk boundary — queueing ft1 behind
            # both weights cost a ~4us PE stall at the first boundary
            ftX1 = io_pool_early.tile([128, KG * NF], FT_DT, tag="ftX",
                                      name="ftXe_1")
            f2tX1 = io_pool_early.tile([128, KG * NF], MMDT, tag="f2tX",
                                       name="f2tXe_1")
            nc.scalar.dma_start(ftX1[:, 0:HLF], ft[1, :, 0:HLF])
            nc.scalar.dma_start(ftX1[:, HLF:], ft[1, :, HLF:])
            wv2_t = load_weight(wv2)

            bn1s_t = bnv_t[:, 0 * MG : 1 * MG]
            bn1b_t = bnv_t[:, 1 * MG : 2 * MG]
            bn2s_t = bnv_t[:, 2 * MG : 3 * MG]
            bn2b_t = bnv_t[:, 3 * MG : 4 * MG]
            zero_t = bnv_t[:, 4 * MG : 5 * MG]
            ident_t = bnv_t[:, 5 * MG : 5 * MG + 128]

            # f2t of chunk 1 last: its H stage runs a full chunk later
            nc.scalar.dma_start(f2tX1[:, 0:HLF], f2t[1, :, 0:HLF])
            nc.scalar.dma_start(f2tX1[:, HLF:], f2t[1, :, HLF:])
            early_loads[1] = (ftX1, f2tX1)

            io_pool = ctx.enter_context(tc.tile_pool(name="io", bufs=2))
            act_pool = ctx.enter_context(tc.tile_pool(name="acts", bufs=2))
            psA = ctx.enter_context(tc.tile_pool(name="psA", bufs=4, space="PSUM"))
            psB = ctx.enter_context(tc.tile_pool(name="psB", bufs=3, space="PSUM"))
            psM = ctx.enter_context(tc.tile_pool(name="psM", bufs=1, space="PSUM"))
            # weight-observer target (write-once columns)
            mb = psM.tile([128, 16], FP32, tag="mb")

            # HAM warm-up: tiny matmuls on a zeroed scratch tile while the
            # first weight/activation DMAs are in flight.  The PE's
            # clock-ramp activity window is free-running, so touching it
            # during the otherwise-idle DMA wait moves the 1.2->2.4GHz
            # ramp off the first real matmuls of chunk 0 (which measured
            # 375-580ns instead of 216ns).
            warm = consts.tile([128, 128], FP8, tag="warm")
            nc.gpsimd.memset(warm[:], 0.0)
            prev_w = None
            for _ in range(32):
                wmm = nc.tensor.matmul(
                    mb[:, 14:16], lhsT=warm[:, 0:128], rhs=warm[:, 0:2],
                    start=True, stop=True, skip_group_check=True).ins
                if prev_w is not None:
                    add_dep_helper(wmm, prev_w, sync=False, reason="warm")
                prev_w = wmm

            ALU = mybir.AluOpType

            DR = mybir.MatmulPerfMode.DoubleRow

            def mm_stage(pool, w_t, src_t, dst_t, mo, func,
                         scale=1.0, bias=0.0, after=None, act_after=None,
                         dve_evict=False, mode=None):
                ps = pool.tile([128, NF], FP32, tag="ps")
                last = None
                if mode == "oneop":
                    # fp8 DoubleRow, k-planes = (hi, lo) weight split,
                    # rhs read twice via a stride-0 broadcast
                    for k in range(KG):
                        rhs = src_t[:, ds(k * NF, NF)].unsqueeze(
                            1).broadcast_to((128, 2, NF))
                        mm = nc.tensor.matmul(
                            ps[:],
                            lhsT=w_t[:, k, :, ds(mo * 128, 128)],
                            rhs=rhs, start=(k == 0), stop=(k == KG - 1),
                            perf_mode=DR)
                        last = mm.ins
                        if after is not None:
                            add_dep_helper(mm.ins, after, sync=False,
                                           reason="mm order")
                elif mode == "half":
                    # k0-1: one pair-DR matmul, both operands single fp8;
                    # k2-3: hi/lo oneop matmuls
                    rhs = src_t[:, ds(0, 2 * NF)].rearrange(
                        "p (two n) -> p two n", two=2)
                    mm = nc.tensor.matmul(
                        ps[:], lhsT=wk8_t[:, :, ds(mo * 128, 128)],
                        rhs=rhs, start=True, stop=False, perf_mode=DR)
                    last = mm.ins
                    if after is not None:
                        add_dep_helper(mm.ins, after, sync=False,
                                       reason="mm order")
                    for k in (2, 3):
                        rhs = src_t[:, ds(k * NF, NF)].unsqueeze(
                            1).broadcast_to((128, 2, NF))
                        mm = nc.tensor.matmul(
                            ps[:],
                            lhsT=w_t[:, k, :, ds(mo * 128, 128)],
                            rhs=rhs, start=False, stop=(k == KG - 1),
                            perf_mode=DR)
                        last = mm.ins
                        if after is not None:
                            add_dep_helper(mm.ins, after, sync=False,
                                           reason="mm order")
                elif mode == "pair":
                    # fp8 DoubleRow over k-group pairs, both operands fp8
                    for g in range(0, KG, 2):
                        rhs = src_t[:, ds(g * NF, 2 * NF)].rearrange(
                            "p (two n) -> p two n", two=2)
                        mm = nc.tensor.matmul(
                            ps[:],
                            lhsT=w_t[:, g:g + 2, ds(mo * 128, 128)],
                            rhs=rhs, start=(g == 0), stop=(g == KG - 2),
                            perf_mode=DR)
                        last = mm.ins
                        if after is not None:
                            add_dep_helper(mm.ins, after, sync=False,
                                           reason="mm order")
                else:
                    for k in range(KG):
                        mm = nc.tensor.matmul(
                            ps[:],
                            lhsT=w_t[:, k, ds(mo * 128, 128)],
                            rhs=src_t[:, ds(k * NF, NF)],
                            start=(k == 0),
                            stop=(k == KG - 1),
                        )
                        last = mm.ins
                        if after is not None:
                            add_dep_helper(mm.ins, after, sync=False,
                                           reason="mm order")
                dst = dst_t[:, ds(mo * NF, NF)]
                if dve_evict:
                    # relu (+ bias / x32 de-scale) eviction on the DVE to
                    # offload the saturated ACT engine; BN scale is folded
                    # into the weights host-side
                    if not isinstance(bias, float):
                        act = nc.vector.tensor_scalar(
                            dst, ps[:], bias, 0.0, ALU.add, ALU.max)
                    elif scale == 1.0:
                        act = nc.vector.tensor_scalar_max(dst, ps[:], 0.0)
                    else:
                        act = nc.vector.tensor_scalar(
                            dst, ps[:], scale, 0.0, ALU.mult, ALU.max)
                else:
                    act = nc.scalar.activation(
                        dst, ps[:], func, scale=scale, bias=bias)
                if act_after is not None:
                    add_dep_helper(act.ins, act_after, sync=False,
                                   reason="act order")
                return last, act.ins, ps

            state = {}

            for j in range(NCHUNK):
                s = {}
                s2 = state.get(j - 2, {})
                s1 = state.get(j - 1, {})

                # ---- loads: two half-DMAs per tensor ----
                if j < 2:
                    ftX, f2tX = early_loads[j]
                    lds = ()
                else:
                    ftX = io_pool_early.tile([128, KG * NF], FT_DT,
                                             tag="ftX", name=f"ftX_{j}")
                    f2tX = io_pool_early.tile([128, KG * NF], MMDT,
                                              tag="f2tX", name=f"f2tX_{j}")
                    lds = (
                        nc.scalar.dma_start(ftX[:, 0:HLF], ft[j, :, 0:HLF]),
                        nc.scalar.dma_start(ftX[:, HLF:], ft[j, :, HLF:]),
                        nc.scalar.dma_start(f2tX[:, 0:HLF],
                                            f2t[j, :, 0:HLF]),
                        nc.scalar.dma_start(f2tX[:, HLF:], f2t[j, :, HLF:]),
                    )
                for d in lds:
                    if "ao_last" in s1:
                        add_dep_helper(d.ins, s1["ao_last"], sync=False,
                                       reason="loads after prev ao")

                p_t = act_pool.tile([128, MG * NF], MMDT, tag="P")
                p8_t = (act_pool.tile([128, MG * NF], FP8, tag="P8",
                                      name=f"p8_{j}")
                        if C_FP8 else None)
                h_t = act_pool.tile([128, MG * NF], MMDT, tag="H")
                # bf16 V/C/O: ACT evictions round to bf16, the DVE
                # combine gets 2x (16-bit) throughput, out-DMA halves
                v_t = act_pool.tile([128, MG * NF], MMDT, tag="V")
                c_t = act_pool.tile([128, MG * NF], MMDT, tag="Cc")
                o_t = io_pool.tile([128, MG * NF], MMDT, tag="O")

                # ---- PE observers ----
                # pe_afterA gates the P stage, pe_afterB (emitted between
                # the P and H loops) gates H/C/V.  Emitting B late lets
                # the P matmuls start while the previous chunk's last V
                # eviction (B's wait) is still in flight.
                def wobs_ap(lhsT, rhs, i, prev=None):
                    ob = nc.tensor.matmul(
                        mb[:, ds(2 * i, 2)],
                        lhsT=lhsT, rhs=rhs, start=True, stop=True).ins
                    if prev is not None:
                        add_dep_helper(ob, prev, sync=False,
                                       reason="wobs order")
                    return ob

                def wobs(w_t, i, prev=None):
                    return wobs_ap(w_t[:, 0, 0:128], w_t[:, 0, 0:2], i,
                                   prev)

                if j == 0:
                    if P_ONEOP and P_HALF:
                        ob = wobs_ap(wk8_t[:, 0, 0:128],
                                     wk8_t[:, 0, 0:2], 0)
                        ob = wobs_ap(wk_t[:, 2, 0, 0:128],
                                     wk_t[:, 2, 0, 0:2], 4, prev=ob)
                        pe_afterA = wobs_ap(wk_t[:, 2, 1, 0:128],
                                            wk_t[:, 2, 1, 0:2], 5, prev=ob)
                    elif P_ONEOP:
                        # both wk DMAs (hi and lo planes) must be observed
                        # before the first P matmul reads them
                        ob = wobs_ap(wk_t[:, 0, 0, 0:128],
                                     wk_t[:, 0, 0, 0:2], 0)
                        pe_afterA = wobs_ap(wk_t[:, 0, 1, 0:128],
                                            wk_t[:, 0, 1, 0:2], 4, prev=ob)
                    else:
                        pe_afterA = wobs(wk_t, 0)
                else:
                    ps0 = psA.tile([128, NF], FP32, tag="ps")
                    obsA = nc.tensor.matmul(
                        ps0[0:128, 0:2],
                        lhsT=s1["c_t"][:, ds(3 * NF, 128)],
                        rhs=s1["c_t"][:, ds(3 * NF, 2)],
                        start=True, stop=True).ins
                    pe_afterA = obsA

                # ---- ACT observers ----
                if j == 0:
                    ao_last = ao_bnv
                else:
                    ao1 = nc.scalar.activation(
                        scrA[0:1, ds(3 * j, 1)],
                        s1["o_t"][0:1, 0:1], AF.Copy).ins
                    ao2 = nc.scalar.activation(
                        scrA[0:1, ds(3 * j + 1, 1)],
                        s1["v_t"][0:1, ds(3 * NF, 1)], AF.Copy).ins
                    add_dep_helper(ao2, ao1, sync=False,
                                   reason="act obs order")
                    ao_last = ao2
                s["ao_last"] = ao_last



                # ---- stages ----
                h_last = None
                dve_anchor = s2.get("comb_add")
                if j == 0:
                    # one-time: DVE observed the bnv DMA queue during
                    # startup, before the bias-carrying evictions read it
                    dve_anchor = anchor_bnv
                p_last = None
                for mo in range(MG):
                    mm, act, ps = mm_stage(
                        psA, wk_t, ftX, p_t, mo, AF.Relu,
                        scale=(1.0 / W8SCALE if P_ONEOP else 1.0),
                        after=pe_afterA, act_after=dve_anchor,
                        dve_evict=True,
                        mode=("half" if (P_ONEOP and P_HALF) else
                              "oneop" if P_ONEOP else None))
                    p_last = mm
                    if C_FP8:
                        # fp8 copy for the C-stage rhs, right behind the
                        # bf16 eviction on the DVE: the C matmuls' single
                        # DVE wait (>= p8) then also covers the bf16
                        # evicts and the P psum-bank reuse
                        if P_ONEOP:
                            ev = nc.vector.tensor_scalar(
                                p8_t[:, ds(mo * NF, NF)], ps[:],
                                1.0 / W8SCALE, 0.0, ALU.mult, ALU.max)
                        else:
                            ev = nc.vector.tensor_scalar_max(
                                p8_t[:, ds(mo * NF, NF)], ps[:], 0.0)
                        add_dep_helper(ev.ins, act, sync=False,
                                       reason="p8 after bf16 evict")

                if j == 0:
                    pe_afterB = wobs(wv1_t, 1, prev=p_last)
                else:
                    ps0 = psB.tile([128, NF], FP32, tag="ps")
                    obsB = nc.tensor.matmul(
                        ps0[0:128, 0:2],
                        lhsT=s1["v_t"][:, ds(3 * NF, 128)],
                        rhs=s1["v_t"][:, ds(3 * NF, 2)],
                        start=True, stop=True).ins
                    add_dep_helper(obsB, p_last, sync=False,
                                   reason="obs order")
                    pe_afterB = obsB

                # H evictions split DVE/ACT: with the P stage shortened
                # by P_HALF the DVE became the binding per-chunk resource
                # (~1.7us of V/C stalls); mo2-3 go to ACT, which has
                # slack.  Each V matmul's k-th MM waits only the evictor
                # of its own h-region, so the split keeps one wait per
                # instruction (same-sem waits merge to max tick).
                for mo in range(MG):
                    on_act = mo >= 2
                    mm, act, _ = mm_stage(
                        psB, wv1_t, f2tX, h_t, mo, AF.Relu,
                        bias=bn1b_t[:, mo : mo + 1],
                        after=pe_afterB,
                        act_after=(ao_last if on_act else dve_anchor),
                        dve_evict=not on_act,
                    )
                    h_last = mm

                if j == 0:
                    wobs_c = wobs(wvc_t, 2, prev=h_last)
                c_last = None
                for mo in range(MG):
                    mm, act, _ = mm_stage(psA, wvc_t,
                                          p8_t if C_FP8 else p_t,
                                          c_t, mo, AF.Sigmoid,
                                          scale=(1.0 / W8SCALE if C_FP8
                                                 else 1.0),
                                          bias=zero_t[:, mo : mo + 1],
                                          act_after=ao_last,
                                          mode=("pair" if C_FP8 else None))
                    c_last = mm
                if j == 0:
                    wobs_v = wobs(wv2_t, 3, prev=c_last)
                for mo in range(MG):
                    mm, act, _ = mm_stage(
                        psB, wv2_t, h_t, v_t, mo, AF.Relu,
                        bias=bn2b_t[:, mo : mo + 1],
                        act_after=ao_last,
                    )

                # ---- combine (DVE) ----
                do1 = None
                if "out_dma" in s2:
                    do1 = nc.vector.tensor_copy(
                        o_t[0:1, 0:1], scrD[0:1, ds(j - 1, 1)]).ins
                    if "comb_add" in s1:
                        add_dep_helper(do1, s1["comb_add"], sync=False,
                                       reason="do1 after j-1 combine")
                if MM_DTYPE == "float32r":
                    p_f32 = p_t[:].bitcast(FP32)
                else:
                    # DVE converts on read; bf16 P costs ~0.4% extra
                    # rounding on the P*C term only
                    p_f32 = p_t[:]
                out_view = outs[j][:].rearrange("(m p) n -> p m n", p=128)
                if j == NCHUNK - 1:
                    # pipeline the final chunk per mo-group so the tail
                    # (evict -> combine -> out-DMA) overlaps instead of
                    # serializing after the last matmul
                    prev_op = do1
                    for mo in range(MG):
                        sl = ds(mo * NF, NF)
                        oc = nc.vector.tensor_copy(
                            scrD[0:1, ds(2 * NCHUNK + 2 + 2 * mo, 1)],
                            c_t[0:1, ds(mo * NF, 1)]).ins
                        if prev_op is not None:
                            add_dep_helper(oc, prev_op, sync=False,
                                           reason="tail order")
                        cm = nc.vector.tensor_mul(
                            o_t[:, sl], p_f32[:, sl], c_t[:, sl])
                        add_dep_helper(cm.ins, oc, sync=False,
                                       reason="tail order")
                        ov = nc.vector.tensor_copy(
                            scrD[0:1, ds(2 * NCHUNK + 3 + 2 * mo, 1)],
                            v_t[0:1, ds(mo * NF, 1)]).ins
                        add_dep_helper(ov, cm.ins, sync=False,
                                       reason="tail order")
                        ca = nc.vector.tensor_add(
                            o_t[:, sl], o_t[:, sl], v_t[:, sl])
                        add_dep_helper(ca.ins, ov, sync=False,
                                       reason="tail order")
                        ao3 = nc.scalar.activation(
                            scrA[0:1, ds(3 * j + 2, 1)] if mo == 0 else
                            scrA[0:1, ds(3 * NCHUNK + mo, 1)],
                            o_t[0:1, ds(mo * NF + 1, 1)], AF.Copy).ins
                        od = nc.scalar.dma_start(out_view[:, mo, :],
                                                 o_t[:, sl])
                        add_dep_helper(od.ins, ao3, sync=False,
                                       reason="od after ao3")
                        prev_op = ca.ins
                    s["comb_add"] = prev_op
                    s["out_dma"] = od.ins
                else:
                    # DVE observes C3's ACT tick, multiplies, then observes
                    # V3's tick and adds — the mul doesn't idle until the
                    # V evictions land
                    do2a = nc.vector.tensor_copy(
                        scrD[0:1, ds(j, 1)], c_t[0:1, ds(3 * NF, 1)]).ins
                    if do1 is not None:
                        add_dep_helper(do2a, do1, sync=False,
                                       reason="dve order")
                    cm = nc.vector.tensor_mul(o_t[:], p_f32, c_t[:])
                    add_dep_helper(cm.ins, do2a, sync=False,
                                   reason="mul order")
                    do2b = nc.vector.tensor_copy(
                        scrD[0:1, ds(NCHUNK + 1 + j, 1)],
                        v_t[0:1, ds(3 * NF, 1)]).ins
                    add_dep_helper(do2b, cm.ins, sync=False,
                                   reason="dve order")
                    ca = nc.vector.tensor_add(o_t[:], o_t[:], v_t[:])
                    add_dep_helper(ca.ins, do2b, sync=False,
                                   reason="add order")
                    s["comb_add"] = ca.ins

                    # ACT observes this chunk's combine so the (ACT-issued)
                    # out-DMA needs no fresh DVE wait of its own
                    ao3 = nc.scalar.activation(
                        scrA[0:1, ds(3 * j + 2, 1)], o_t[0:1, 1:2],
                        AF.Copy).ins
                    od = nc.scalar.dma_start(
                        out_view,
                        o_t[:].rearrange("p (m n) -> p m n", m=MG),
                    )
                    add_dep_helper(od.ins, ao3, sync=False,
                                   reason="od after ao3")
                    s["out_dma"] = od.ins
                s["p_t"] = p_t
                s["c_t"] = c_t
                s["v_t"] = v_t
                s["o_t"] = o_t
                state[j] = s

    return nc


def _mm_np_dtype():
    if MM_DTYPE == "bfloat16":
        import ml_dtypes

        return ml_dtypes.bfloat16
    return np.float32


def _prep_inputs(inputs):
    MM_NP = _mm_np_dtype()
    f = np.asarray(inputs["features"], np.float32)
    f2 = np.asarray(inputs["features2"], np.float32)

    def wT(name):
        return np.ascontiguousarray(np.asarray(inputs[name], np.float32).T)

    wk_h, wv1_h, wv2_h, wvc_h = wT("Wk"), wT("Wv1"), wT("Wv2"), wT("Wvc")

    def bn_inv(pre):
        g = np.asarray(inputs[f"{pre}_gamma"], np.float32)
        v = np.asarray(inputs[f"{pre}_var"], np.float32)
        return g / np.sqrt(v + BN_EPS)

    # fold the BN scale into the weight columns (bias stays separate)
    wv1_h = np.ascontiguousarray(wv1_h * bn_inv("bn1")[None, :])
    wv2_h = np.ascontiguousarray(wv2_h * bn_inv("bn2")[None, :])

    def bn_fold(pre):
        g = np.asarray(inputs[f"{pre}_gamma"], np.float32)
        b = np.asarray(inputs[f"{pre}_beta"], np.float32)
        m = np.asarray(inputs[f"{pre}_mean"], np.float32)
        v = np.asarray(inputs[f"{pre}_var"], np.float32)
        inv = g / np.sqrt(v + BN_EPS)
        shift = b - m * inv
        to_tile = lambda x: np.ascontiguousarray(x.reshape(MG, 128).T)
        return to_tile(inv), to_tile(shift)

    bn1s_h, bn1b_h = bn_fold("bn1")
    bn2s_h, bn2b_h = bn_fold("bn2")
    bnv_h = np.ascontiguousarray(np.concatenate(
        [bn1s_h, bn1b_h, bn2s_h, bn2b_h,
         np.zeros((128, MG), np.float32),
         np.zeros((128, 128), np.float32)],
        axis=1,
    ))

    shared = {
        "wv1": wv1_h.astype(MM_NP),
        "wv2": wv2_h.astype(MM_NP),
        "bnv": bnv_h,
    }
    if P_ONEOP:
        import ml_dtypes

        F8NP = ml_dtypes.float8_e4m3fn
        wk32 = (W8SCALE * wk_h).astype(np.float32)
        hi = wk32.astype(F8NP)
        lo = (wk32 - hi.astype(np.float32)).astype(F8NP)
        shared["wkhi"] = np.ascontiguousarray(hi)
        shared["wklo"] = np.ascontiguousarray(lo)
        if P_HALF:
            shared["wk8"] = np.ascontiguousarray(
                (W8SCALE * wk_h[0:C // 2]).astype(np.float32).astype(F8NP))
        f = f.astype(F8NP)
    else:
        shared["wk"] = wk_h.astype(MM_NP)
        f = f.astype(MM_NP)
    if C_FP8:
        import ml_dtypes

        F8NP = ml_dtypes.float8_e4m3fn
        shared["wvc"] = np.ascontiguousarray(
            (W8SCALE * wvc_h).astype(np.float32).astype(F8NP))
    else:
        shared["wvc"] = wvc_h.astype(MM_NP)
    f2 = f2.astype(MM_NP)

    def chunked_T(x):  # [NS, C] rows -> [NCHUNK, 128, KG*NF] k-major
        t = x.T  # [C, NS]; c = k*128 + p
        a = t.reshape(KG, 128, NCHUNK, NF)
        return np.ascontiguousarray(
            a.transpose(2, 1, 0, 3).reshape(NCHUNK, 128, KG * NF))

    in_maps = []
    for i in range(N_CORES):
        rows = slice(i * NS, (i + 1) * NS)
        in_maps.append({
            "ft": chunked_T(f[rows]),
            "f2t": chunked_T(f2[rows]),
            **shared,
        })
    return in_maps


def _gather_out(res_map):
    """[NCHUNK x [D, NF]] per-core outputs -> [NS, D] rows."""
    chunks = [np.asarray(res_map[f"out{j}"], np.float32)
              for j in range(NCHUNK)]
    stacked = np.stack(chunks, axis=0)          # [NCHUNK, D, NF]
    return stacked.transpose(0, 2, 1).reshape(NS, D)


def run(inputs, trace=False):
    from concourse.bass_utils import run_bass_kernel_spmd

    if "nc" not in _CACHE:
        _CACHE["nc"] = _build_program()
    nc = _CACHE["nc"]

    in_maps = _prep_inputs(inputs)
    res = run_bass_kernel_spmd(
        nc, in_maps, list(range(N_CORES)), trace=trace
    )
    full = np.concatenate(
        [_gather_out(r) for r in res.results], axis=0
    ).astype(np.float32)
    return full, res


def kernel(**inputs) -> np.ndarray:
    out, _ = run(inputs, trace=False)
    return out

